# revision 40
# baseline (speedup 1.0000x reference)
"""Trainium2 Bass kernel: per-row InstanceNorm + Linear(512->512) + ReLU.

Computes, for x [N, 512], W [512, 512], b [512]:
    xn = (x - mean_row) * rsqrt(var_row + 1e-5)      (biased var, per row)
    y  = relu(xn @ W.T + b)

Strategy: data-parallel over rows across 8 NeuronCores, bf16 I/O both ways
(halves DMA vs fp32). Host-side preprocessing (layout/dtype only, plus tiny
weight algebra):
  - x cast to bf16.
  - Wc = W - rowmean(W): then (x*rstd) @ Wc.T == ((x-mean)*rstd) @ W.T
    exactly, so the device never needs the row means.
  - output returned transposed (yT [512, rows]) in bf16; host transposes
    back and casts to fp32.

Device pipeline, software-pipelined over 512-row supertiles (ST=4 tiles of
128 rows) processed in 2-supertile groups:
  stage A (5 supertiles ahead): DMA-in -> 4x bn_stats -> 4x bn_aggr
           -> ACT sqrt(var+eps) -> DVE reciprocal
           -> 4x D = I*rstd (DVE tensor_scalar, bf16 diag matrix)
  stage T: 4x4 PE "scaled transposes" (regular matmul x_chunk.T @ D fuses
           the normalization scale and keeps HAM warm), pairs of tiles into
           one 2-bank psum tile -> single FD=1024 ACT evac each
  mains:   per oc-pair, 16 matmuls vs host-prepped WcT accumulate BOTH
           supertiles of the group into [128, 2, 512] psum tiles, so each
           relu+bias evac is one FD=1024 op with a single per-partition
           bias (3 on ACT, 1 on DVE per group). The next group's stage T is
           emitted between the two oc-pair blocks so the PE fills the
           psum-release stall. Output DMAs are issued from the idle GPSIMD
           sequencer — a waiting out-DMA at the head of the in-order Sync
           queue would block later input-DMA issues.

Measured on 8 axon trn2 cores: HW exec ~270 us (baseline 354 us): PE ~224,
ACT ~192, DVE ~192, DMA ~150+ us. Max scale-relative error ~5.4e-3 (bf16
input quantization dominates; gate 2e-2).
"""

import os
import sys

import numpy as np

sys.path.insert(0, "/opt/trn_rl_repo")

import ml_dtypes  # noqa: E402

import concourse.bacc as bacc  # noqa: E402
import concourse.bass as bass  # noqa: E402
import concourse.tile as tile  # noqa: E402
from concourse import mybir  # noqa: E402
from concourse.bass_utils import run_bass_kernel_spmd  # noqa: E402

N_CORES = 8
N_FULL = 200000
N_IN = 512
N_OUT = 512
P = 128
KC = N_IN // P  # 4 contraction chunks
OC = N_OUT // P  # 4 output chunks
ST = 4  # row-tiles per supertile (512 rows)
ROWS_PER_CORE = 25088  # 49 supertiles of 512; 8*25088 = 200704 >= 200000
N_PAD = ROWS_PER_CORE * N_CORES

EPS = 1e-5

F32 = mybir.dt.float32
BF16 = mybir.dt.bfloat16

# Tuning knobs (env-overridable for quick A/B on hardware)
STATS_DT = BF16 if os.environ.get("K_STATS_BF16", "1") == "1" else F32
# How many of the 4 yT-output evacuations go to the DVE (rest on ACT)
YT_EVAC_DVE = int(os.environ.get("K_YT_DVE", "1"))
# How many of the 4 xsT-transpose evacuations go to the DVE (rest on ACT)
XS_EVAC_DVE = int(os.environ.get("K_XS_DVE", "0"))

LAST_RUN = None  # BassKernelResults of the most recent run (for test harness)


def build_bass(rows_per_core: int) -> bass.Bass:
    rows_per_st = P * ST
    nst = rows_per_core // rows_per_st
    assert rows_per_core % rows_per_st == 0

    nc = bacc.Bacc()
    x_d = nc.declare_dram_parameter("x", [rows_per_core, N_IN], BF16, isOutput=False)
    wt_d = nc.declare_dram_parameter("wt", [N_IN, N_OUT], BF16, isOutput=False)
    b_d = nc.declare_dram_parameter("bvec", [P, OC], F32, isOutput=False)
    ident_d = nc.declare_dram_parameter("ident", [P, P], BF16, isOutput=False)
    yt_d = nc.declare_dram_parameter("yt", [N_OUT, rows_per_core], BF16, isOutput=True)

    with tile.TileContext(nc) as tc:
        with (
            tc.tile_pool(name="singles", bufs=1) as singles,
            tc.tile_pool(name="xin", bufs=8) as xin_pool,
            tc.tile_pool(name="stats", bufs=20) as stats_pool,
            tc.tile_pool(name="dmat", bufs=24) as d_pool,
            tc.tile_pool(name="xnt", bufs=5) as xnt_pool,
            tc.tile_pool(name="yout", bufs=3) as y_pool,
            tc.tile_pool(name="pst", bufs=2, space="PSUM") as pst_pool,
            tc.tile_pool(name="psy", bufs=2, space="PSUM") as psy_pool,
        ):
            # --- constants (loaded once) ---
            # wt_sb[p, c, o] = Wc.T[c*128+p, o]
            wt_sb = singles.tile([P, KC, N_OUT], BF16)
            nc.sync.dma_start(out=wt_sb, in_=wt_d[:, :].rearrange("(c p) o -> p c o", p=P))
            ident_sb = singles.tile([P, P], BF16)
            nc.sync.dma_start(out=ident_sb, in_=ident_d[:, :])
            bias_sb = singles.tile([P, OC], F32)  # bias_sb[p, oc] = b[oc*128+p]
            nc.sync.dma_start(out=bias_sb, in_=b_d[:, :])
            eps_sb = singles.tile([P, 1], F32)
            nc.vector.memset(eps_sb, EPS)

            # supertile s, partition p, sub-tile j  <->  row s*512 + p*ST + j
            x_b = x_d[:, :].rearrange("(s p j) i -> s p j i", p=P, j=ST)
            # yt_d[o, col]: col = s*512 + n; pair supertiles so each output
            # descriptor covers 2 adjacent 512-col blocks (2KB runs).
            y_flat = yt_d[:, :].rearrange("(oc p) col -> p oc col", p=P)

            PST = P * ST

            def stage_a(s):
                """DMA-in + stats + D matrices for supertile s."""
                xb = xin_pool.tile([P, ST, N_IN], BF16)
                nc.sync.dma_start(out=xb, in_=x_b[s])
                st6 = stats_pool.tile([P, ST, 6], STATS_DT)
                for j in range(ST):
                    nc.vector.bn_stats(out=st6[:, j, :], in_=xb[:, j, :])
                mv = stats_pool.tile([P, ST, 2], F32)
                for j in range(ST):
                    nc.vector.bn_aggr(out=mv[:, j, :], in_=st6[:, j, :])
                # sd = sqrt(var + eps) for all ST tiles in one ACT op
                sd = stats_pool.tile([P, ST], F32)
                nc.scalar.activation(
                    out=sd, in_=mv[:, :, 1],
                    func=mybir.ActivationFunctionType.Sqrt,
                    bias=eps_sb[:, :], scale=1.0,
                )
                rstd = stats_pool.tile([P, ST], F32)
                nc.vector.reciprocal(out=rstd, in_=sd)
                dmats = []
                for j in range(ST):
                    dmat = d_pool.tile([P, P], BF16)
                    nc.vector.tensor_scalar(
                        out=dmat, in0=ident_sb[:, :],
                        scalar1=rstd[:, j:j + 1], scalar2=None,
                        op0=mybir.AluOpType.mult,
                    )
                    dmats.append(dmat)
                return xb, dmats

            groups = [(g, min(2, nst - g)) for g in range(0, nst, 2)]
            LOOKAHEAD = 5  # supertiles of stats/load emitted ahead of compute
            staged = {}
            for s in range(min(LOOKAHEAD, nst)):
                staged[s] = stage_a(s)

            def stage_t(s):
                """Scaled transposes for supertile s: xsT[i,n] = x[n,i]*rstd[n]
                (regular matmul vs D=I*rstd keeps PE HAM-warm). Generator:
                yields after each tile's 4 transpose matmuls so the caller can
                interleave them between main-matmul blocks — a transpose's
                LDWEIGHTS (107 ns) only streams 53 ns, so back-to-back
                transposes are load-bound unless hidden under main streams."""
                if s + LOOKAHEAD < nst:
                    staged[s + LOOKAHEAD] = stage_a(s + LOOKAHEAD)
                xb, dmats = staged.pop(s)
                xnt = xnt_pool.tile([P, KC, PST], BF16)  # [i, c, n(4 tiles)]
                xnts[s] = xnt
                for jp in range(ST // 2):
                    # two tiles' transposes into one 2-bank psum tile so
                    # the evacuation is a single FD=1024 op
                    ps_t2 = pst_pool.tile([P, 2, KC, P], F32)
                    for jj in range(2):
                        j = jp * 2 + jj
                        for c in range(KC):
                            nc.tensor.matmul(
                                ps_t2[:, jj, c, :],
                                xb[:, j, c * P:(c + 1) * P],  # lhsT [n, i]
                                dmats[j][:, :],                # rhs  [n, n]
                                start=True, stop=True,
                            )
                        yield
                    # evac psum->sbuf: [i, jj, c, n'] -> xnt[:, c, (jp*2+jj)*128+n']
                    dst = xnt[:, :, jp * 2 * P:(jp * 2 + 2) * P].rearrange(
                        "p c (jj nn) -> p jj c nn", jj=2
                    )
                    if jp < XS_EVAC_DVE:
                        nc.vector.tensor_copy(dst, ps_t2[:, :, :, :])
                    else:
                        nc.scalar.copy(dst, ps_t2[:, :, :, :])

            xnts = {}
            for s in range(min(2, nst)):
                for _ in stage_t(s):
                    pass
            for gi, (g, gsz) in enumerate(groups):
                yb = y_pool.tile([P, OC, 2 * PST], BF16)
                # next group's transposes, pumped one tile at a time between
                # main-matmul blocks so their LDWEIGHTS hide under main streams
                gens = []
                if gi + 1 < len(groups):
                    ng, ngsz = groups[gi + 1]
                    gens = [stage_t(ng + nk) for nk in range(ngsz)]

                def pump():
                    while gens:
                        try:
                            next(gens[0])
                            return
                        except StopIteration:
                            gens.pop(0)

                # --- main matmuls: yT[oc] = WcT[:,oc].T @ xsT (+bias, relu).
                # Both supertiles of the group accumulate the same oc into one
                # 2-bank psum tile, so each relu+bias evac is FD=1024 with a
                # single per-partition bias. oc-pairs sequence to fit PSUM.
                for ocp in range(OC // 2):
                    psys = [
                        psy_pool.tile([P, 2, PST], F32, name="psyd", tag="psyd")
                        for _ in range(2)
                    ]
                    for k in range(gsz):
                        for jo in range(2):
                            oc = ocp * 2 + jo
                            for c in range(KC):
                                nc.tensor.matmul(
                                    psys[jo][:, k, :],
                                    wt_sb[:, c, oc * P:(oc + 1) * P],  # lhsT [i, o]
                                    xnts[g + k][:, c, :],               # rhs  [i, n]
                                    start=(c == 0),
                                    stop=(c == KC - 1),
                                )
                            pump()
                    for jo in range(2):
                        oc = ocp * 2 + jo
                        out_ap = yb[:, oc, :gsz * PST].rearrange(
                            "p (k n) -> p k n", k=gsz
                        )
                        if ocp == 0 and jo < YT_EVAC_DVE:
                            # one per group on DVE to balance ACT load
                            nc.vector.tensor_scalar(
                                out=out_ap, in0=psys[jo][:, :gsz, :],
                                scalar1=bias_sb[:, oc:oc + 1], scalar2=0.0,
                                op0=mybir.AluOpType.add, op1=mybir.AluOpType.max,
                            )
                        else:
                            nc.scalar.activation(
                                out=out_ap, in_=psys[jo][:, :gsz, :],
                                func=mybir.ActivationFunctionType.Relu,
                                bias=bias_sb[:, oc:oc + 1], scale=1.0,
                            )
                while gens:
                    pump()
                for k in range(gsz):
                    del xnts[g + k]
                # issue output DMAs from the (idle) GPSIMD sequencer: a
                # waiting out-DMA at the head of the Sync queue would
                # otherwise block all later input-DMA issues (in-order queue)
                nc.gpsimd.dma_start(
                    out=y_flat[:, :, g * PST:(g + gsz) * PST],
                    in_=yb[:, :, :gsz * PST],
                )
    nc.compile()
    return nc


_BASS_CACHE: dict[int, bass.Bass] = {}


def _get_bass(rows_per_core: int) -> bass.Bass:
    if rows_per_core not in _BASS_CACHE:
        _BASS_CACHE[rows_per_core] = build_bass(rows_per_core)
    return _BASS_CACHE[rows_per_core]


def _run(x_pad: np.ndarray, W: np.ndarray, b: np.ndarray, rows_per_core: int) -> np.ndarray:
    """x_pad: [n_cores*rows_per_core, 512] bf16. Returns [n_cores*rows, 512] f32."""
    global LAST_RUN
    nc = _get_bass(rows_per_core)
    # center W rows so the matmul on (x*rstd) implements the mean subtraction
    Wc = W - W.mean(axis=1, keepdims=True)
    wt = np.ascontiguousarray(Wc.T).astype(ml_dtypes.bfloat16)
    bb = np.ascontiguousarray(b.reshape(OC, P).T).astype(np.float32)  # [P, OC]
    ident = np.eye(P, dtype=ml_dtypes.bfloat16)
    in_maps = [
        {
            "x": np.ascontiguousarray(x_pad[c * rows_per_core:(c + 1) * rows_per_core]),
            "wt": wt,
            "bvec": bb,
            "ident": ident,
        }
        for c in range(N_CORES)
    ]
    trace = bool(os.environ.get("BASS_TRACE"))
    res = run_bass_kernel_spmd(nc, in_maps, list(range(N_CORES)), trace=trace)
    LAST_RUN = res
    # yt: [512, rows_per_core] bf16 per core. Device column s*512 + j*128 + p
    # holds row s*512 + p*ST + j (interleaved DMA layout): unpermute, then
    # transpose to [rows, 512] and cast to f32.
    nst = rows_per_core // (P * ST)
    outs = []
    for c in range(N_CORES):
        yt = np.asarray(res.results[c]["yt"])  # [512, rows] bf16
        y = yt.reshape(N_OUT, nst, ST, P).transpose(1, 3, 2, 0)  # [s, p, j, o]
        outs.append(y.reshape(rows_per_core, N_OUT).astype(np.float32))
    return np.concatenate(outs, axis=0)


def kernel(x: np.ndarray, W: np.ndarray, b: np.ndarray) -> np.ndarray:
    x = np.asarray(x, dtype=np.float32)
    W = np.asarray(W, dtype=np.float32)
    b = np.asarray(b, dtype=np.float32)
    n = x.shape[0]
    x_pad = np.zeros((N_PAD, N_IN), dtype=ml_dtypes.bfloat16)
    x_pad[:n] = x.astype(ml_dtypes.bfloat16)
    y_pad = _run(x_pad, W, b, ROWS_PER_CORE)
    return np.ascontiguousarray(y_pad[:n])
